# revision 8
# baseline (speedup 1.0000x reference)
"""MidiGPT dense transformer forward pass on 8 Trainium2 NeuronCores.

Sharding: DP2 x TP4. Cores 0-3 process batch 0, cores 4-7 batch 1.
Within a 4-core group: attention heads are split 2-per-core, FFN hidden
dim split 4-way. Per layer: AllGather of attention head outputs
(o^T, [256,2048] -> [1024,2048]) then replicated proj; AllReduce of the
FFN partial outputs. LayerNorm gains/biases are folded into the
following matmul weights on the host. All matmuls run as float32r.
"""
import sys
sys.path.insert(0, "/opt/trn_rl_repo")
import numpy as np

import concourse.bass as bass
import concourse.bacc as bacc
import concourse.tile as tile
from concourse import mybir
from concourse import bass_utils

# Model dims
L, H = 4, 8
E = 1024
DH = 128           # head size
B, T = 2, 2048
FF = 4 * E         # 4096
P_SZ, V_SZ = 130, 128
NEP, NEV = 1014, 8
OUT_D = 1 + 1 + P_SZ + V_SZ   # 260

N_CORES = 8
TP = 4             # tensor-parallel group size
HPC = H // TP      # heads per core = 2
DHC = HPC * DH     # head dims per core = 256
FFC = FF // TP     # ffn hidden per core = 1024
TB = 512           # token block size
NTB = T // TB      # 4 token blocks
NTC = T // 128     # 16 token chunks
NEC = E // 128     # 8 embedding chunks
GROUPS = [[0, 1, 2, 3], [4, 5, 6, 7]]

F32 = mybir.dt.float32
F32R = mybir.dt.float32r
SCALE = DH ** -0.5


def _r(ap):
    return ap.bitcast(F32R)


def build_kernel():
    nc = bacc.Bacc(trn_type="TRN2", target_bir_lowering=False, debug=False,
                   num_devices=N_CORES)

    # ---- per-core DRAM inputs ----
    x0 = nc.dram_tensor("x0", [T, E], F32, kind="ExternalInput")
    wqkv = nc.dram_tensor("wqkv", [L, E, 3 * DHC], F32, kind="ExternalInput")
    wproj = nc.dram_tensor("wproj", [L, E, E], F32, kind="ExternalInput")
    w1 = nc.dram_tensor("w1", [L, E, FFC], F32, kind="ExternalInput")
    w2 = nc.dram_tensor("w2", [L, FFC, E], F32, kind="ExternalInput")
    whead = nc.dram_tensor("whead", [E, OUT_D], F32, kind="ExternalInput")
    y = nc.dram_tensor("y", [TB, OUT_D], F32, kind="ExternalOutput")

    with tile.TileContext(nc) as tc:
        _build_body(nc, tc, x0, wqkv, wproj, w1, w2, whead, y)
    nc.compile()
    return nc


def _build_body(nc, tc, x0, wqkv, wproj, w1, w2, whead, y):
    from contextlib import ExitStack
    ctx = ExitStack()
    pool = lambda name, bufs, space="SBUF": ctx.enter_context(
        tc.tile_pool(name=name, bufs=bufs, space=space))

    consts = pool("consts", 1)
    xp = pool("x", 16)
    hp = pool("h", 2)
    arp = pool("arin", 2)
    yp = pool("y", 4)
    dram = pool("dram", 2, space="DRAM")

    ps_t = pool("ps_t", 2, space="PSUM")     # transposes
    ps_m = pool("ps_m", 4, space="PSUM")     # general matmul accum
    ps_d = pool("ps_d", 2, space="PSUM")     # denominators

    # constants
    ones32 = consts.tile([128, 1], F32)
    nc.vector.memset(ones32, 1.0)
    ones = consts.tile([128, 1], F32R)
    nc.vector.tensor_copy(ones[:], ones32[:])
    eps = consts.tile([128, 1], F32)
    nc.vector.memset(eps, 1e-5)
    ident = consts.tile([128, 128], F32)
    from concourse.masks import make_identity
    make_identity(nc, ident)

    # ---- load x ----
    x = []
    for tc_i in range(NTC):
        xt = xp.tile([128, E], F32, name=f"x{tc_i}", tag="x")
        nc.sync.dma_start(xt[:], x0[128 * tc_i:128 * (tc_i + 1), :])
        x.append(xt)

    def layernorm_tile(src, dst_pool, name):
        """LN over free dim (E) of a [128, E] tile -> new tile."""
        st = dst_pool.tile([128, 2, 6], F32, name=f"{name}_st", tag="ln_st")
        nc.vector.bn_stats(st[:, 0, :], src[:, 0:512])
        nc.vector.bn_stats(st[:, 1, :], src[:, 512:1024])
        mv = dst_pool.tile([128, 2], F32, name=f"{name}_mv", tag="ln_mv")
        nc.vector.bn_aggr(mv[:], st[:])
        rstd = dst_pool.tile([128, 1], F32, name=f"{name}_rs", tag="ln_rs")
        nc.scalar.activation(rstd[:], mv[:, 1:2],
                             mybir.ActivationFunctionType.Sqrt, bias=eps[:])
        nc.vector.reciprocal(rstd[:], rstd[:])
        out = dst_pool.tile([128, E], F32, name=name, tag="ln_out")
        nc.vector.tensor_scalar(out[:], src[:], scalar1=mv[:, 0:1],
                                scalar2=rstd[:],
                                op0=mybir.AluOpType.subtract,
                                op1=mybir.AluOpType.mult)
        return out

    def transpose_block(htp, h_tiles, j, name):
        """Transpose 4 [128,E] token-chunk tiles (block j) -> 8 [128,TB] tiles."""
        out = []
        for e in range(NEC):
            ht = htp.tile([128, TB], F32R, name=f"{name}_e{e}", tag="hT")
            out.append(ht)
        for ti, h in enumerate(h_tiles):
            for e in range(NEC):
                pt = ps_t.tile([128, 128], F32, name="ps_tr", tag="ps_tr")
                nc.tensor.transpose(pt[:], h[:, 128 * e:128 * (e + 1)], ident[:])
                nc.any.tensor_copy(out[e][:, 128 * ti:128 * (ti + 1)], pt[:])
        return out

    for l in range(L):
        with (
            tc.tile_pool(name="qt", bufs=4) as qtp,
            tc.tile_pool(name="v", bufs=16) as vp,
        ):
            # qT/kT: [dh 128 per head][T]; v: token-major [128, DHC]/chunk
            qT = [qtp.tile([128, T], F32R, name=f"qT{h}", tag="qkT")
                  for h in range(HPC)]
            kT = [qtp.tile([128, T], F32R, name=f"kT{h}", tag="qkT")
                  for h in range(HPC)]
            v = [vp.tile([128, DHC], F32R, name=f"v{t}", tag="v")
                 for t in range(NTC)]

            with (
                tc.tile_pool(name="wq", bufs=8) as wqp,
                tc.tile_pool(name="hT", bufs=12) as htp,
            ):
                wq_t = []
                for e in range(NEC):
                    wt = wqp.tile([128, 3 * DHC], F32R, name=f"wq{e}", tag="wq")
                    nc.sync.dma_start(wt[:], _r(wqkv[l, 128 * e:128 * (e + 1), :]))
                    wq_t.append(wt)
                for j in range(NTB):
                    h_tiles = [layernorm_tile(x[4 * j + ti], hp,
                                              f"h{l}_{4*j+ti}")
                               for ti in range(4)]
                    hT = transpose_block(htp, h_tiles, j, f"hT{l}_{j}")
                    # qT/kT: out[m 128, t TB] accum e; m in {q0,q1,k0,k1}
                    for m in range(4):
                        pm = ps_m.tile([128, TB], F32, name="ps_qk",
                                       tag="ps_mm")
                        for e in range(NEC):
                            nc.tensor.matmul(
                                pm[:], _r(wq_t[e][:, 128 * m:128 * (m + 1)]),
                                _r(hT[e][:]), start=(e == 0),
                                stop=(e == NEC - 1))
                        dst = (qT[m] if m < HPC else kT[m - HPC])
                        nc.any.tensor_copy(dst[:, TB * j:TB * (j + 1)], pm[:])
                    # v token-major: out[t 128, dh DHC] accum e
                    for ti in range(4):
                        t_i = 4 * j + ti
                        pm = ps_m.tile([128, DHC], F32, name="ps_v",
                                       tag="ps_mm")
                        for e in range(NEC):
                            nc.tensor.matmul(
                                pm[:], _r(hT[e][:, 128 * ti:128 * (ti + 1)]),
                                _r(wq_t[e][:, 2 * DHC:3 * DHC]),
                                start=(e == 0), stop=(e == NEC - 1))
                        nc.any.tensor_copy(v[t_i][:], pm[:])

            # ---- attention per head ----
            ag_in = dram.tile([DHC, T], F32, name=f"agin{l}", tag="agin")
            ag_out = dram.tile([TP * DHC, T], F32, name=f"agout{l}",
                               tag="agout")
            with (
                tc.tile_pool(name="oT", bufs=2) as otp,
                tc.tile_pool(name="es", bufs=4) as esp,
                tc.tile_pool(name="den", bufs=2) as denp,
            ):
                oT = [otp.tile([128, T], F32, name=f"oT{h}", tag="oT")
                      for h in range(HPC)]
                for h in range(HPC):
                    for j in range(NTB):
                        po = ps_m.tile([128, TB], F32, name="ps_o",
                                       tag="ps_mm")
                        pd = ps_d.tile([1, TB], F32, name="ps_den",
                                       tag="ps_den")
                        nchunks = 4 * j + 4
                        for i in range(nchunks):
                            ps = ps_m.tile([128, TB], F32, name="ps_s",
                                           tag="ps_mm")
                            nc.tensor.matmul(
                                ps[:], _r(kT[h][:, 128 * i:128 * (i + 1)]),
                                _r(qT[h][:, TB * j:TB * (j + 1)]),
                                start=True, stop=True)
                            es = esp.tile([128, TB], F32R, name="es", tag="es")
                            nc.scalar.activation(
                                es[:], ps[:],
                                mybir.ActivationFunctionType.Exp, scale=SCALE)
                            if 128 * i >= TB * j:   # diagonal: causal mask
                                nc.gpsimd.affine_select(
                                    out=es[:], in_=es[:],
                                    compare_op=mybir.AluOpType.is_ge,
                                    fill=0.0, base=TB * j - 128 * i,
                                    pattern=[[1, TB]], channel_multiplier=-1)
                            nc.tensor.matmul(
                                po[:], _r(v[i][:, DH * h:DH * (h + 1)]),
                                _r(es[:]), start=(i == 0),
                                stop=(i == nchunks - 1))
                            nc.tensor.matmul(
                                pd[:], _r(ones[:]), _r(es[:]),
                                start=(i == 0), stop=(i == nchunks - 1))
                        den = denp.tile([1, TB], F32, name="den", tag="den")
                        nc.vector.reciprocal(den[:], pd[:])
                        denb = denp.tile([128, TB], F32, name="denb",
                                         tag="denb")
                        nc.gpsimd.partition_broadcast(denb[:], den[:])
                        nc.vector.tensor_mul(oT[h][:, TB * j:TB * (j + 1)],
                                             po[:], denb[:])
                # ---- AllGather o across the TP group ----
                for h in range(HPC):
                    nc.sync.dma_start(ag_in[128 * h:128 * (h + 1), :], oT[h][:])
            nc.gpsimd.collective_compute(
                "AllGather", mybir.AluOpType.bypass, replica_groups=GROUPS,
                ins=[ag_in[:].opt()], outs=[ag_out[:].opt()])

        # ---- proj (replicated) + residual ----
        with (
            tc.tile_pool(name="oF", bufs=8) as ofp,
            tc.tile_pool(name="wproj", bufs=8) as wpp,
        ):
            oF = []
            for e in range(NEC):
                of = ofp.tile([128, T], F32R, name=f"of{e}", tag="oF")
                nc.sync.dma_start(of[:], _r(ag_out[128 * e:128 * (e + 1), :]))
                oF.append(of)
            wp_t = []
            for e in range(NEC):
                wt = wpp.tile([128, E], F32R, name=f"wp{e}", tag="wp")
                nc.sync.dma_start(wt[:], _r(wproj[l, 128 * e:128 * (e + 1), :]))
                wp_t.append(wt)
            for t_i in range(NTC):
                for n in range(2):
                    pm = ps_m.tile([128, TB], F32, name="ps_pr", tag="ps_mm")
                    for e in range(NEC):
                        nc.tensor.matmul(
                            pm[:], _r(oF[e][:, 128 * t_i:128 * (t_i + 1)]),
                            _r(wp_t[e][:, TB * n:TB * (n + 1)]),
                            start=(e == 0), stop=(e == NEC - 1))
                    nc.vector.tensor_add(x[t_i][:, TB * n:TB * (n + 1)],
                                         x[t_i][:, TB * n:TB * (n + 1)],
                                         pm[:])

        # ---- FFN ----
        ar_in = dram.tile([T, E], F32, name=f"arin{l}", tag="arin")
        ar_out = dram.tile([T, E], F32, name=f"arout{l}", tag="arout")
        with (
            tc.tile_pool(name="w1", bufs=8) as w1p,
            tc.tile_pool(name="w2", bufs=8) as w2p,
            tc.tile_pool(name="hT2", bufs=12) as htp2,
            tc.tile_pool(name="aT", bufs=8) as atp,
            tc.tile_pool(name="delta", bufs=2) as dlp,
        ):
            w1_t = []
            w2_t = []
            for e in range(NEC):
                wt = w1p.tile([128, FFC], F32R, name=f"w1{e}", tag="w1")
                nc.sync.dma_start(wt[:], _r(w1[l, 128 * e:128 * (e + 1), :]))
                w1_t.append(wt)
            for f in range(FFC // 128):
                wt = w2p.tile([128, E], F32R, name=f"w2{f}", tag="w2")
                nc.sync.dma_start(wt[:], _r(w2[l, 128 * f:128 * (f + 1), :]))
                w2_t.append(wt)
            for j in range(NTB):
                h2_tiles = [layernorm_tile(x[4 * j + ti], hp, f"g{l}_{4*j+ti}")
                            for ti in range(4)]
                h2T = transpose_block(htp2, h2_tiles, j, f"h2T{l}_{j}")
                # a^T[f 128, t TB] = relu(W1^T @ h2^T)
                aT = []
                for m in range(FFC // 128):
                    pm = ps_m.tile([128, TB], F32, name="ps_a", tag="ps_mm")
                    for e in range(NEC):
                        nc.tensor.matmul(
                            pm[:], _r(w1_t[e][:, 128 * m:128 * (m + 1)]),
                            _r(h2T[e][:]), start=(e == 0),
                            stop=(e == NEC - 1))
                    at = atp.tile([128, TB], F32R, name=f"aT{m}", tag="aT")
                    nc.scalar.activation(at[:], pm[:],
                                         mybir.ActivationFunctionType.Relu)
                    aT.append(at)
                # delta[t, E] partial = a @ W2c
                for ti in range(4):
                    t_i = 4 * j + ti
                    dl = dlp.tile([128, E], F32, name="dl", tag="dl")
                    for n in range(2):
                        pm = ps_m.tile([128, TB], F32, name="ps_w2",
                                       tag="ps_mm")
                        for f in range(FFC // 128):
                            nc.tensor.matmul(
                                pm[:], _r(aT[f][:, 128 * ti:128 * (ti + 1)]),
                                _r(w2_t[f][:, TB * n:TB * (n + 1)]),
                                start=(f == 0), stop=(f == FFC // 128 - 1))
                        nc.any.tensor_copy(dl[:, TB * n:TB * (n + 1)], pm[:])
                    nc.sync.dma_start(ar_in[128 * t_i:128 * (t_i + 1), :],
                                      dl[:])

        nc.gpsimd.collective_compute(
            "AllReduce", mybir.AluOpType.add, replica_groups=GROUPS,
            ins=[ar_in[:].opt()], outs=[ar_out[:].opt()])
        for t_i in range(NTC):
            ai = arp.tile([128, E], F32, name="ai", tag="ai")
            nc.sync.dma_start(ai[:], ar_out[128 * t_i:128 * (t_i + 1), :])
            nc.vector.tensor_add(x[t_i][:], x[t_i][:], ai[:])

    # ---- final LN + heads (all token blocks; host slices per batch) ----
    yfull = nc.dram_tensor("yfull", [T, OUT_D], F32, kind="ExternalOutput")
    with (
        tc.tile_pool(name="whp", bufs=8) as whp,
        tc.tile_pool(name="hTf", bufs=12) as htpf,
    ):
        wh_t = []
        for e in range(NEC):
            wt = whp.tile([128, OUT_D], F32R, name=f"wh{e}", tag="wh")
            nc.sync.dma_start(wt[:], _r(whead[128 * e:128 * (e + 1), :]))
            wh_t.append(wt)
        for j in range(NTB):
            hf_tiles = [layernorm_tile(x[4 * j + ti], hp, f"f{4*j+ti}")
                        for ti in range(4)]
            hfT = transpose_block(htpf, hf_tiles, j, f"hfT{j}")
            for ti in range(4):
                t_i = 4 * j + ti
                pm = ps_m.tile([128, OUT_D], F32, name="ps_y", tag="ps_mm")
                for e in range(NEC):
                    nc.tensor.matmul(
                        pm[:], _r(hfT[e][:, 128 * ti:128 * (ti + 1)]),
                        _r(wh_t[e][:]), start=(e == 0), stop=(e == NEC - 1))
                yt = yp.tile([128, OUT_D], F32, name="yt", tag="yt")
                nc.any.tensor_copy(yt[:], pm[:])
                nc.sync.dma_start(yfull[128 * t_i:128 * (t_i + 1), :], yt[:])
    # y (small output) kept for interface stability
    nc.sync.dma_start(y[:], yfull[0:TB, :])
    ctx.close()


# ------------------------------------------------------------------
# host side
# ------------------------------------------------------------------
_NC_CACHE = None


def _get_nc():
    global _NC_CACHE
    if _NC_CACHE is None:
        _NC_CACHE = build_kernel()
    return _NC_CACHE


def kernel(cxt, pitch_emb, vel_emb, Wq, Wk, Wv, proj_w, proj_b,
           ln1_g, ln1_b, ln2_g, ln2_b, W1, b1, W2, b2, lnf_g, lnf_b,
           time_w, time_b, dur_w, dur_b, pitch_w, pitch_b, vel_w, vel_b):
    cxt = np.asarray(cxt, dtype=np.float32)
    pid = cxt[..., 2].astype(np.int32)
    vid = cxt[..., 3].astype(np.int32)
    x0 = np.concatenate([cxt[..., 0:2],
                         np.asarray(pitch_emb)[pid],
                         np.asarray(vel_emb)[vid]], axis=-1)  # [B,T,E]

    # fold LN gains/biases into weights (biases are zero in practice but
    # fold generally; resulting matmul-bias terms are added on host only
    # if nonzero -- here they are zero, asserted below)
    Wq, Wk, Wv = np.asarray(Wq), np.asarray(Wk), np.asarray(Wv)
    proj_w, W1, W2 = np.asarray(proj_w), np.asarray(W1), np.asarray(W2)
    ln1_g, ln1_b = np.asarray(ln1_g), np.asarray(ln1_b)
    ln2_g, ln2_b = np.asarray(ln2_g), np.asarray(ln2_b)
    lnf_g, lnf_b = np.asarray(lnf_g), np.asarray(lnf_b)
    for nm, bias in (("ln1_b", ln1_b), ("ln2_b", ln2_b), ("lnf_b", lnf_b),
                     ("proj_b", proj_b), ("b1", b1), ("b2", b2),
                     ("time_b", time_b), ("dur_b", dur_b),
                     ("pitch_b", pitch_b), ("vel_b", vel_b)):
        assert not np.any(np.asarray(bias)), f"nonzero bias {nm} unsupported"

    Wq_f = ln1_g[:, :, None] * Wq    # [L,E,E]
    Wk_f = ln1_g[:, :, None] * Wk
    Wv_f = ln1_g[:, :, None] * Wv
    W1_f = ln2_g[:, :, None] * W1    # [L,E,4E]
    whead = np.concatenate([np.asarray(time_w), np.asarray(dur_w),
                            np.asarray(pitch_w), np.asarray(vel_w)], axis=1)
    whead_f = (lnf_g[:, None] * whead).astype(np.float32)  # [E,260]

    in_maps = []
    for c in range(N_CORES):
        b, tp = c // TP, c % TP
        hsl = slice(DH * HPC * tp, DH * HPC * (tp + 1))   # 256 head dims
        fsl = slice(FFC * tp, FFC * (tp + 1))             # 1024 ffn dims
        wqkv_c = np.concatenate(
            [Wq_f[:, :, hsl], Wk_f[:, :, hsl], Wv_f[:, :, hsl]],
            axis=2).astype(np.float32)                    # [L,E,768]
        in_maps.append({
            "x0": np.ascontiguousarray(x0[b], dtype=np.float32),
            "wqkv": np.ascontiguousarray(wqkv_c),
            "wproj": np.ascontiguousarray(proj_w.astype(np.float32)),
            "w1": np.ascontiguousarray(W1_f[:, :, fsl].astype(np.float32)),
            "w2": np.ascontiguousarray(W2[:, fsl, :].astype(np.float32)),
            "whead": whead_f,
        })

    global _LAST_IN_MAPS
    _LAST_IN_MAPS = in_maps
    nc = _get_nc()
    res = bass_utils.run_bass_kernel_spmd(
        nc, in_maps, core_ids=list(range(N_CORES)))
    out = np.zeros((B, T, OUT_D), dtype=np.float32)
    out[0] = res.results[0]["yfull"]
    out[1] = res.results[4]["yfull"]
    return out


# revision 9
# speedup vs baseline: 1.2639x; 1.2639x over previous
"""MidiGPT dense transformer forward pass on 8 Trainium2 NeuronCores.

Sharding: DP2 x TP4. Cores 0-3 process batch 0, cores 4-7 batch 1.
Within a 4-core group: attention heads are split 2-per-core, FFN hidden
dim split 4-way. Per layer: AllGather of attention head outputs
(o^T, [256,2048] -> [1024,2048]) then replicated proj; AllReduce of the
FFN partial outputs. LayerNorm gains/biases are folded into the
following matmul weights on the host. All matmuls run as float32r.
"""
import sys
sys.path.insert(0, "/opt/trn_rl_repo")
import numpy as np
import ml_dtypes

import concourse.bass as bass
import concourse.bacc as bacc
import concourse.tile as tile
from concourse import mybir
from concourse import bass_utils

# Model dims
L, H = 4, 8
E = 1024
DH = 128           # head size
B, T = 2, 2048
FF = 4 * E         # 4096
P_SZ, V_SZ = 130, 128
NEP, NEV = 1014, 8
OUT_D = 1 + 1 + P_SZ + V_SZ   # 260

N_CORES = 8
TP = 4             # tensor-parallel group size
HPC = H // TP      # heads per core = 2
DHC = HPC * DH     # head dims per core = 256
FFC = FF // TP     # ffn hidden per core = 1024
TB = 512           # token block size
NTB = T // TB      # 4 token blocks
NTC = T // 128     # 16 token chunks
NEC = E // 128     # 8 embedding chunks
GROUPS = [[0, 1, 2, 3], [4, 5, 6, 7]]

F32 = mybir.dt.float32
F32R = mybir.dt.float32r
BF16 = mybir.dt.bfloat16
SCALE = DH ** -0.5


def _r(ap):
    return ap.bitcast(F32R)


def build_kernel():
    nc = bacc.Bacc(trn_type="TRN2", target_bir_lowering=False, debug=False,
                   num_devices=N_CORES)

    # ---- per-core DRAM inputs ----
    x0 = nc.dram_tensor("x0", [T, E], F32, kind="ExternalInput")
    wqkv = nc.dram_tensor("wqkv", [L, E, 3 * DHC], F32, kind="ExternalInput")
    wproj = nc.dram_tensor("wproj", [L, E, E], BF16, kind="ExternalInput")
    w1 = nc.dram_tensor("w1", [L, E, FFC], F32, kind="ExternalInput")
    w2 = nc.dram_tensor("w2", [L, FFC, E], F32, kind="ExternalInput")
    whead = nc.dram_tensor("whead", [E, OUT_D], F32, kind="ExternalInput")
    y = nc.dram_tensor("y", [TB, OUT_D], F32, kind="ExternalOutput")

    with tile.TileContext(nc) as tc:
        _build_body(nc, tc, x0, wqkv, wproj, w1, w2, whead, y)
    nc.compile()
    return nc


def _build_body(nc, tc, x0, wqkv, wproj, w1, w2, whead, y):
    from contextlib import ExitStack
    ctx = ExitStack()
    pool = lambda name, bufs, space="SBUF": ctx.enter_context(
        tc.tile_pool(name=name, bufs=bufs, space=space))

    consts = pool("consts", 1)
    xp = pool("x", 16)
    hp = pool("h", 2)
    arp = pool("arin", 2)
    yp = pool("y", 4)
    dram = pool("dram", 2, space="DRAM")

    ps_t = pool("ps_t", 2, space="PSUM")     # transposes
    ps_m = pool("ps_m", 4, space="PSUM")     # general matmul accum
    ps_d = pool("ps_d", 2, space="PSUM")     # denominators

    # constants
    ones32 = consts.tile([128, 1], F32)
    nc.vector.memset(ones32, 1.0)
    ones = consts.tile([128, 1], F32R)
    nc.vector.tensor_copy(ones[:], ones32[:])
    eps = consts.tile([128, 1], F32)
    nc.vector.memset(eps, 1e-5)
    ident = consts.tile([128, 128], F32)
    from concourse.masks import make_identity
    make_identity(nc, ident)

    # ---- load x ----
    x = []
    for tc_i in range(NTC):
        xt = xp.tile([128, E], F32, name=f"x{tc_i}", tag="x")
        nc.sync.dma_start(xt[:], x0[128 * tc_i:128 * (tc_i + 1), :])
        x.append(xt)

    def layernorm_tile(src, dst_pool, name):
        """LN over free dim (E) of a [128, E] tile -> new tile."""
        st = dst_pool.tile([128, 2, 6], F32, name=f"{name}_st", tag="ln_st")
        nc.vector.bn_stats(st[:, 0, :], src[:, 0:512])
        nc.vector.bn_stats(st[:, 1, :], src[:, 512:1024])
        mv = dst_pool.tile([128, 2], F32, name=f"{name}_mv", tag="ln_mv")
        nc.vector.bn_aggr(mv[:], st[:])
        rstd = dst_pool.tile([128, 1], F32, name=f"{name}_rs", tag="ln_rs")
        nc.scalar.activation(rstd[:], mv[:, 1:2],
                             mybir.ActivationFunctionType.Sqrt, bias=eps[:])
        nc.vector.reciprocal(rstd[:], rstd[:])
        out = dst_pool.tile([128, E], F32, name=name, tag="ln_out")
        nc.vector.tensor_scalar(out[:], src[:], scalar1=mv[:, 0:1],
                                scalar2=rstd[:],
                                op0=mybir.AluOpType.subtract,
                                op1=mybir.AluOpType.mult)
        return out

    def transpose_block(htp, h_tiles, j, name):
        """Transpose 4 [128,E] token-chunk tiles (block j) -> 8 [128,TB] tiles."""
        out = []
        for e in range(NEC):
            ht = htp.tile([128, TB], F32R, name=f"{name}_e{e}", tag="hT")
            out.append(ht)
        for ti, h in enumerate(h_tiles):
            for e in range(NEC):
                pt = ps_t.tile([128, 128], F32, name="ps_tr", tag="ps_tr")
                nc.tensor.transpose(pt[:], h[:, 128 * e:128 * (e + 1)], ident[:])
                nc.any.tensor_copy(out[e][:, 128 * ti:128 * (ti + 1)], pt[:])
        return out

    for l in range(L):
        with (
            tc.tile_pool(name="qt", bufs=4) as qtp,
            tc.tile_pool(name="v", bufs=16) as vp,
        ):
            # qT/kT: [dh 128 per head][T]; v: token-major [128, DHC]/chunk
            qT = [qtp.tile([128, T], F32R, name=f"qT{h}", tag="qkT")
                  for h in range(HPC)]
            kT = [qtp.tile([128, T], F32R, name=f"kT{h}", tag="qkT")
                  for h in range(HPC)]
            v = [vp.tile([128, DHC], F32R, name=f"v{t}", tag="v")
                 for t in range(NTC)]

            with (
                tc.tile_pool(name="wq", bufs=8) as wqp,
                tc.tile_pool(name="hT", bufs=12) as htp,
            ):
                wq_t = []
                for e in range(NEC):
                    wt = wqp.tile([128, 3 * DHC], F32R, name=f"wq{e}", tag="wq")
                    nc.sync.dma_start(wt[:], _r(wqkv[l, 128 * e:128 * (e + 1), :]))
                    wq_t.append(wt)
                for j in range(NTB):
                    h_tiles = [layernorm_tile(x[4 * j + ti], hp,
                                              f"h{l}_{4*j+ti}")
                               for ti in range(4)]
                    hT = transpose_block(htp, h_tiles, j, f"hT{l}_{j}")
                    # qT/kT: out[m 128, t TB] accum e; m in {q0,q1,k0,k1}
                    for m in range(4):
                        pm = ps_m.tile([128, TB], F32, name="ps_qk",
                                       tag="ps_mm")
                        for e in range(NEC):
                            nc.tensor.matmul(
                                pm[:], _r(wq_t[e][:, 128 * m:128 * (m + 1)]),
                                _r(hT[e][:]), start=(e == 0),
                                stop=(e == NEC - 1))
                        dst = (qT[m] if m < HPC else kT[m - HPC])
                        nc.any.tensor_copy(dst[:, TB * j:TB * (j + 1)], pm[:])
                    # v token-major: out[t 128, dh DHC] accum e
                    for ti in range(4):
                        t_i = 4 * j + ti
                        pm = ps_m.tile([128, DHC], F32, name="ps_v",
                                       tag="ps_mm")
                        for e in range(NEC):
                            nc.tensor.matmul(
                                pm[:], _r(hT[e][:, 128 * ti:128 * (ti + 1)]),
                                _r(wq_t[e][:, 2 * DHC:3 * DHC]),
                                start=(e == 0), stop=(e == NEC - 1))
                        nc.any.tensor_copy(v[t_i][:], pm[:])

            # ---- attention per head ----
            ag_in = dram.tile([DHC, T], BF16, name=f"agin{l}", tag="agin")
            ag_out = dram.tile([TP * DHC, T], BF16, name=f"agout{l}",
                               tag="agout")
            with (
                tc.tile_pool(name="oT", bufs=2) as otp,
                tc.tile_pool(name="es", bufs=4) as esp,
                tc.tile_pool(name="den", bufs=2) as denp,
            ):
                oT = [otp.tile([128, T], BF16, name=f"oT{h}", tag="oT")
                      for h in range(HPC)]
                for h in range(HPC):
                    for j in range(NTB):
                        po = ps_m.tile([128, TB], F32, name="ps_o",
                                       tag="ps_mm")
                        pd = ps_d.tile([1, TB], F32, name="ps_den",
                                       tag="ps_den")
                        nchunks = 4 * j + 4
                        for i in range(nchunks):
                            ps = ps_m.tile([128, TB], F32, name="ps_s",
                                           tag="ps_mm")
                            nc.tensor.matmul(
                                ps[:], _r(kT[h][:, 128 * i:128 * (i + 1)]),
                                _r(qT[h][:, TB * j:TB * (j + 1)]),
                                start=True, stop=True)
                            es = esp.tile([128, TB], F32R, name="es", tag="es")
                            nc.scalar.activation(
                                es[:], ps[:],
                                mybir.ActivationFunctionType.Exp, scale=SCALE)
                            if 128 * i >= TB * j:   # diagonal: causal mask
                                nc.gpsimd.affine_select(
                                    out=es[:], in_=es[:],
                                    compare_op=mybir.AluOpType.is_ge,
                                    fill=0.0, base=TB * j - 128 * i,
                                    pattern=[[1, TB]], channel_multiplier=-1)
                            nc.tensor.matmul(
                                po[:], _r(v[i][:, DH * h:DH * (h + 1)]),
                                _r(es[:]), start=(i == 0),
                                stop=(i == nchunks - 1))
                            nc.tensor.matmul(
                                pd[:], _r(ones[:]), _r(es[:]),
                                start=(i == 0), stop=(i == nchunks - 1))
                        den = denp.tile([1, TB], F32, name="den", tag="den")
                        nc.vector.reciprocal(den[:], pd[:])
                        denb = denp.tile([128, TB], F32, name="denb",
                                         tag="denb")
                        nc.gpsimd.partition_broadcast(denb[:], den[:])
                        nc.vector.tensor_mul(oT[h][:, TB * j:TB * (j + 1)],
                                             po[:], denb[:])
                # ---- AllGather o across the TP group ----
                for h in range(HPC):
                    nc.sync.dma_start(ag_in[128 * h:128 * (h + 1), :], oT[h][:])
            nc.gpsimd.collective_compute(
                "AllGather", mybir.AluOpType.bypass, replica_groups=GROUPS,
                ins=[ag_in[:].opt()], outs=[ag_out[:].opt()])

        # ---- proj (replicated) + residual ----
        with (
            tc.tile_pool(name="oF", bufs=8) as ofp,
            tc.tile_pool(name="wproj", bufs=8) as wpp,
        ):
            oF = []
            for e in range(NEC):
                of = ofp.tile([128, T], BF16, name=f"of{e}", tag="oF")
                nc.sync.dma_start(of[:], ag_out[128 * e:128 * (e + 1), :])
                oF.append(of)
            wp_t = []
            for e in range(NEC):
                wt = wpp.tile([128, E], BF16, name=f"wp{e}", tag="wp")
                nc.sync.dma_start(wt[:], wproj[l, 128 * e:128 * (e + 1), :])
                wp_t.append(wt)
            for t_i in range(NTC):
                for n in range(2):
                    pm = ps_m.tile([128, TB], F32, name="ps_pr", tag="ps_mm")
                    for e in range(NEC):
                        nc.tensor.matmul(
                            pm[:], oF[e][:, 128 * t_i:128 * (t_i + 1)],
                            wp_t[e][:, TB * n:TB * (n + 1)],
                            start=(e == 0), stop=(e == NEC - 1))
                    nc.vector.tensor_add(x[t_i][:, TB * n:TB * (n + 1)],
                                         x[t_i][:, TB * n:TB * (n + 1)],
                                         pm[:])

        # ---- FFN ----
        ar_in = dram.tile([T, E], BF16, name=f"arin{l}", tag="arin")
        ar_out = dram.tile([T, E], BF16, name=f"arout{l}", tag="arout")
        with (
            tc.tile_pool(name="w1", bufs=8) as w1p,
            tc.tile_pool(name="w2", bufs=8) as w2p,
            tc.tile_pool(name="hT2", bufs=12) as htp2,
            tc.tile_pool(name="aT", bufs=8) as atp,
            tc.tile_pool(name="delta", bufs=2) as dlp,
        ):
            w1_t = []
            w2_t = []
            for e in range(NEC):
                wt = w1p.tile([128, FFC], F32R, name=f"w1{e}", tag="w1")
                nc.sync.dma_start(wt[:], _r(w1[l, 128 * e:128 * (e + 1), :]))
                w1_t.append(wt)
            for f in range(FFC // 128):
                wt = w2p.tile([128, E], F32R, name=f"w2{f}", tag="w2")
                nc.sync.dma_start(wt[:], _r(w2[l, 128 * f:128 * (f + 1), :]))
                w2_t.append(wt)
            for j in range(NTB):
                h2_tiles = [layernorm_tile(x[4 * j + ti], hp, f"g{l}_{4*j+ti}")
                            for ti in range(4)]
                h2T = transpose_block(htp2, h2_tiles, j, f"h2T{l}_{j}")
                # a^T[f 128, t TB] = relu(W1^T @ h2^T)
                aT = []
                for m in range(FFC // 128):
                    pm = ps_m.tile([128, TB], F32, name="ps_a", tag="ps_mm")
                    for e in range(NEC):
                        nc.tensor.matmul(
                            pm[:], _r(w1_t[e][:, 128 * m:128 * (m + 1)]),
                            _r(h2T[e][:]), start=(e == 0),
                            stop=(e == NEC - 1))
                    at = atp.tile([128, TB], F32R, name=f"aT{m}", tag="aT")
                    nc.scalar.activation(at[:], pm[:],
                                         mybir.ActivationFunctionType.Relu)
                    aT.append(at)
                # delta[t, E] partial = a @ W2c
                for ti in range(4):
                    t_i = 4 * j + ti
                    dl = dlp.tile([128, E], BF16, name="dl", tag="dl")
                    for n in range(2):
                        pm = ps_m.tile([128, TB], F32, name="ps_w2",
                                       tag="ps_mm")
                        for f in range(FFC // 128):
                            nc.tensor.matmul(
                                pm[:], _r(aT[f][:, 128 * ti:128 * (ti + 1)]),
                                _r(w2_t[f][:, TB * n:TB * (n + 1)]),
                                start=(f == 0), stop=(f == FFC // 128 - 1))
                        nc.any.tensor_copy(dl[:, TB * n:TB * (n + 1)], pm[:])
                    nc.sync.dma_start(ar_in[128 * t_i:128 * (t_i + 1), :],
                                      dl[:])

        nc.gpsimd.collective_compute(
            "AllReduce", mybir.AluOpType.add, replica_groups=GROUPS,
            ins=[ar_in[:].opt()], outs=[ar_out[:].opt()])
        for t_i in range(NTC):
            ai = arp.tile([128, E], BF16, name="ai", tag="ai")
            nc.sync.dma_start(ai[:], ar_out[128 * t_i:128 * (t_i + 1), :])
            nc.vector.tensor_add(x[t_i][:], x[t_i][:], ai[:])

    # ---- final LN + heads (all token blocks; host slices per batch) ----
    yfull = nc.dram_tensor("yfull", [T, OUT_D], F32, kind="ExternalOutput")
    with (
        tc.tile_pool(name="whp", bufs=8) as whp,
        tc.tile_pool(name="hTf", bufs=12) as htpf,
    ):
        wh_t = []
        for e in range(NEC):
            wt = whp.tile([128, OUT_D], F32R, name=f"wh{e}", tag="wh")
            nc.sync.dma_start(wt[:], _r(whead[128 * e:128 * (e + 1), :]))
            wh_t.append(wt)
        for j in range(NTB):
            hf_tiles = [layernorm_tile(x[4 * j + ti], hp, f"f{4*j+ti}")
                        for ti in range(4)]
            hfT = transpose_block(htpf, hf_tiles, j, f"hfT{j}")
            for ti in range(4):
                t_i = 4 * j + ti
                pm = ps_m.tile([128, OUT_D], F32, name="ps_y", tag="ps_mm")
                for e in range(NEC):
                    nc.tensor.matmul(
                        pm[:], _r(hfT[e][:, 128 * ti:128 * (ti + 1)]),
                        _r(wh_t[e][:]), start=(e == 0), stop=(e == NEC - 1))
                yt = yp.tile([128, OUT_D], F32, name="yt", tag="yt")
                nc.any.tensor_copy(yt[:], pm[:])
                nc.sync.dma_start(yfull[128 * t_i:128 * (t_i + 1), :], yt[:])
    # y (small output) kept for interface stability
    nc.sync.dma_start(y[:], yfull[0:TB, :])
    ctx.close()


# ------------------------------------------------------------------
# host side
# ------------------------------------------------------------------
_NC_CACHE = None


def _get_nc():
    global _NC_CACHE
    if _NC_CACHE is None:
        _NC_CACHE = build_kernel()
    return _NC_CACHE


def kernel(cxt, pitch_emb, vel_emb, Wq, Wk, Wv, proj_w, proj_b,
           ln1_g, ln1_b, ln2_g, ln2_b, W1, b1, W2, b2, lnf_g, lnf_b,
           time_w, time_b, dur_w, dur_b, pitch_w, pitch_b, vel_w, vel_b):
    cxt = np.asarray(cxt, dtype=np.float32)
    pid = cxt[..., 2].astype(np.int32)
    vid = cxt[..., 3].astype(np.int32)
    x0 = np.concatenate([cxt[..., 0:2],
                         np.asarray(pitch_emb)[pid],
                         np.asarray(vel_emb)[vid]], axis=-1)  # [B,T,E]

    # fold LN gains/biases into weights (biases are zero in practice but
    # fold generally; resulting matmul-bias terms are added on host only
    # if nonzero -- here they are zero, asserted below)
    Wq, Wk, Wv = np.asarray(Wq), np.asarray(Wk), np.asarray(Wv)
    proj_w, W1, W2 = np.asarray(proj_w), np.asarray(W1), np.asarray(W2)
    ln1_g, ln1_b = np.asarray(ln1_g), np.asarray(ln1_b)
    ln2_g, ln2_b = np.asarray(ln2_g), np.asarray(ln2_b)
    lnf_g, lnf_b = np.asarray(lnf_g), np.asarray(lnf_b)
    for nm, bias in (("ln1_b", ln1_b), ("ln2_b", ln2_b), ("lnf_b", lnf_b),
                     ("proj_b", proj_b), ("b1", b1), ("b2", b2),
                     ("time_b", time_b), ("dur_b", dur_b),
                     ("pitch_b", pitch_b), ("vel_b", vel_b)):
        assert not np.any(np.asarray(bias)), f"nonzero bias {nm} unsupported"

    Wq_f = ln1_g[:, :, None] * Wq    # [L,E,E]
    Wk_f = ln1_g[:, :, None] * Wk
    Wv_f = ln1_g[:, :, None] * Wv
    W1_f = ln2_g[:, :, None] * W1    # [L,E,4E]
    whead = np.concatenate([np.asarray(time_w), np.asarray(dur_w),
                            np.asarray(pitch_w), np.asarray(vel_w)], axis=1)
    whead_f = (lnf_g[:, None] * whead).astype(np.float32)  # [E,260]

    in_maps = []
    for c in range(N_CORES):
        b, tp = c // TP, c % TP
        hsl = slice(DH * HPC * tp, DH * HPC * (tp + 1))   # 256 head dims
        fsl = slice(FFC * tp, FFC * (tp + 1))             # 1024 ffn dims
        wqkv_c = np.concatenate(
            [Wq_f[:, :, hsl], Wk_f[:, :, hsl], Wv_f[:, :, hsl]],
            axis=2).astype(np.float32)                    # [L,E,768]
        in_maps.append({
            "x0": np.ascontiguousarray(x0[b], dtype=np.float32),
            "wqkv": np.ascontiguousarray(wqkv_c),
            "wproj": np.ascontiguousarray(proj_w.astype(ml_dtypes.bfloat16)),
            "w1": np.ascontiguousarray(W1_f[:, :, fsl].astype(np.float32)),
            "w2": np.ascontiguousarray(W2[:, fsl, :].astype(np.float32)),
            "whead": whead_f,
        })

    global _LAST_IN_MAPS
    _LAST_IN_MAPS = in_maps
    nc = _get_nc()
    res = bass_utils.run_bass_kernel_spmd(
        nc, in_maps, core_ids=list(range(N_CORES)))
    out = np.zeros((B, T, OUT_D), dtype=np.float32)
    out[0] = res.results[0]["yfull"]
    out[1] = res.results[4]["yfull"]
    return out
